# revision 1
# baseline (speedup 1.0000x reference)
"""Trainium2 Bass kernel for nn_DiffNet (gnn_message_passing).

The reference's per-element "edge MLP" over the meta stack
(vi, W, vj) -> two 1x1 convs -> weighted sum over the input dim is
linear in its 3 channels, so it collapses algebraically.  With
g = conv1_w.T @ conv2_w[0]  (3 scalars), hb = conv1_b@conv2_w[0]+conv2_b[0],
z = vi @ W.T (no bias), s1[b] = sum_i vi[b,i], s2[b] = sum_i vi[b,i]^2:

    out[b,o] = relu(z+b)[b,o] * (1 + scale*g2*s1[b])
             + scale*(g0*s2[b] + g1*z[b,o] + hb*s1[b])

so the whole network is 3 small matmuls + elementwise, and the problem
is memory-bound on the fc weights (3.5 MB fp32).

Distribution (8 cores, no collectives): fc1/fc2 replicated (any
zero-communication scheme must read them on every core since every
output depends on all of them), fc3 sharded over its output dim
(32 cols/core); full batch B=32 on every core; host concatenates the
8 [32,32] output shards.

On-core layout: activations live transposed [feature(partitions), batch]
in 128-row chunks; weights are passed pre-transposed [in, out] so matmuls
need no on-chip weight transpose.  Matmuls put the (tiny) activation
tile stationary and stream the weight chunk [128, 512] as the moving
operand in float32r (1 cycle/row at N>=512 vs 4 for plain fp32); all
tensors on the matmul dataflow are declared float32r so their producers
satisfy the walrus fp32r-rounding rule.  The z output lands
[batch, out]; a cheap PE transpose brings each 128-col chunk back to
[out, batch] where relu-bias (per-partition), the k1*z term and the
per-batch alpha/beta scalars (broadcast across partitions via a rank-1
ones matmul) are applied with a few wide DVE ops.
"""

import sys

if "/opt/trn_rl_repo" not in sys.path:
    sys.path.insert(0, "/opt/trn_rl_repo")

import numpy as np


def _install_ntff_hook_shim():
    """This image's antenv lacks ``axon_hooks``; bass_utils hard-imports it
    when tracing under axon.  Provide the module and register the ctypes
    NTFF hook from trn_agent_boot so ``trace=True`` yields exec_time_ns."""
    import types

    if "antenv.axon_hooks" in sys.modules:
        return
    try:
        import antenv

        mod = types.ModuleType("antenv.axon_hooks")
        _h = [None]
        mod.set_axon_ntff_profile_hook = lambda hook: _h.__setitem__(0, hook)
        mod.get_axon_ntff_profile_hook = lambda: _h[0]
        sys.modules["antenv.axon_hooks"] = mod
        antenv.axon_hooks = mod
        from trn_agent_boot.trn_boot import _ntff_profile_via_ctypes

        mod.set_axon_ntff_profile_hook(
            _ntff_profile_via_ctypes("/opt/axon/libaxon_pjrt.so")
        )
    except Exception:
        pass


_install_ntff_hook_shim()

N_CORES = 8
B = 32
I1, O1, O2, O3 = 1024, 512, 512, 256
O3L = O3 // N_CORES  # fc3 output cols per core
RATE = 0.1

_CACHE = {}
LAST_RESULTS = None  # BassKernelResults of the most recent run (for test.py)


def _build(k0, k1, k2, kb):
    import concourse.bacc as bacc
    import concourse.mybir as mybir
    import concourse.tile as tile
    import concourse.bass as bass

    f32 = mybir.dt.float32
    f32r = mybir.dt.float32r
    AF = mybir.ActivationFunctionType
    ALU = mybir.AluOpType

    from concourse.tile_rust import add_dep_helper

    nc = bacc.Bacc(
        "TRN2", target_bir_lowering=False, debug=False, num_devices=N_CORES
    )

    f16 = mybir.dt.float16
    # x is f32r (the DMA *rounds* f32r payloads — only matmul operands may
    # travel that way); everything else rides a plain-f32 misc tensor.
    # misc cols: [b12: 0..8) [b3: 8] [eye: 9..41) [onesK: 41]
    # [Kalpha f16 [3,128] packed in f32: 42..106) [Kbeta f16: 106..170)
    XW = 8 * B
    MW = 42 + 128
    xm = nc.declare_dram_parameter("xm", [128, XW], f32r, isOutput=False)
    misc = nc.declare_dram_parameter("misc", [128, MW], f32, isOutput=False)
    w1 = nc.declare_dram_parameter("w1t", [128, 8 * O1], f32r, isOutput=False)
    w2 = nc.declare_dram_parameter("w2t", [128, 4 * O2], f32r, isOutput=False)
    w3 = nc.declare_dram_parameter("w3t", [128, 4 * O3L], f32r, isOutput=False)
    out_d = nc.declare_dram_parameter("out", [O3L, B], f32, isOutput=True)

    with tile.TileContext(nc) as tc:
        with (
            tc.tile_pool(name="wts", bufs=1) as wp,
            tc.tile_pool(name="act", bufs=1) as ap,
            tc.tile_pool(name="ps", bufs=1, space=bass.MemorySpace.PSUM) as pp,
        ):
            tw1 = wp.tile([128, 8 * O1], f32r, tag="w1")
            tw2 = wp.tile([128, 4 * O2], f32r, tag="w2")
            tw3 = wp.tile([128, 4 * O3L], f32r, tag="w3")
            txm = wp.tile([128, XW], f32r, tag="xm")
            tx = txm[:]  # f32r activations for layer 1
            tmisc = wp.tile([128, MW], f32, tag="misc")
            tb12 = tmisc[:, 0:8]
            tb3 = tmisc[0:O3L, 8:9]
            teye = tmisc[0:B, 9:41]
            t1k = tmisc[:, 41:42]  # f32 ones col (K-dir sums)
            # coefficient matrices for the alpha/beta broadcast matmuls:
            # alpha/beta(p, b) = K.T @ s_sb(:, b), sources on rows 0/32/64
            tka16 = tmisc[0:96, 42:106].bitcast(f16)  # [96,128] f16
            tkb16 = tmisc[0:96, 106:170].bitcast(f16)  # [96,128] f16

            # -- DMAs: one HWDGE ring, in need-order, few enough that each
            # gets its own completion-sem lane.  fc1 in thirds so its
            # z-matmuls start as the stream lands.
            nc.sync.dma_start(tmisc[:], misc[:])
            nc.sync.dma_start(txm[:], xm[:])
            for lo, hi in ((0, 3), (3, 6), (6, 8)):
                nc.sync.dma_start(
                    tw1[:, lo * O1 : hi * O1], w1[:, lo * O1 : hi * O1]
                )
            nc.sync.dma_start(tw2[:], w2[:])
            nc.sync.dma_start(tw3[:], w3[:])

            def ordered(dependent, dependency, why):
                if dependent is not None and dependency is not None:
                    add_dep_helper(
                        dependent.ins, dependency.ins, sync=False, reason=why
                    )

            def stats_ab(a_tile, n_c, tag, after_mm=None):
                """a_tile [128, n_c*B] float32r; -> (ab_sb [128, 2*B], bcast).
                ab rows all equal; cols 0:B = alpha(b), B:2B = beta(b).
                Everything on the f32r single-pass path: squares come from a
                DVE multiply writing f32r (the walrus fp32r-producer rule
                allows DVE outputs), so both column-sum chains are f32r."""
                asq = ap.tile([128, n_c * B], f32r, tag=tag + "sq")
                af = a_tile.bitcast(f32)
                nc.vector.tensor_tensor(asq[:], af, af, ALU.mult)
                s1_ps = pp.tile([1, B], f32, tag="s1")
                s2_ps = pp.tile([1, B], f32, tag="s2")
                mm1 = None
                for c in range(n_c):
                    mm = nc.tensor.matmul(
                        s1_ps[:],
                        t1k,
                        af[:, c * B : (c + 1) * B],
                        start=(c == 0),
                        stop=(c == n_c - 1),
                    )
                    mm1 = mm1 or mm
                asqf = asq[:].bitcast(f32)
                for c in range(n_c):
                    nc.tensor.matmul(
                        s2_ps[:],
                        t1k,
                        asqf[:, c * B : (c + 1) * B],
                        start=(c == 0),
                        stop=(c == n_c - 1),
                    )
                ordered(mm1, after_mm, "stats after this layer's z matmuls")
                # engine writes must start at partition 0/32/64 -> spread
                # (s1, s2, 1) over those rows; memset first so junk
                # partitions are finite (their K coefficients are 0) and
                # row 64 is the ones row
                s_sb = ap.tile([96, B], f16, tag=tag + "row")
                nc.vector.memset(s_sb[:], 1.0)
                nc.scalar.copy(s_sb[0:1, :], s1_ps[:])
                nc.scalar.copy(s_sb[32:33, :], s2_ps[:])
                ab_ps = pp.tile([128, 2 * B], f32, tag="ab")
                nc.tensor.matmul(
                    ab_ps[:, 0:B], tka16, s_sb[:], start=True, stop=True
                )
                bcast = nc.tensor.matmul(
                    ab_ps[:, B : 2 * B], tkb16, s_sb[:], start=True, stop=True
                )
                ab_sb = ap.tile([128, 2 * B], f32, tag=tag + "sb")
                nc.scalar.copy(ab_sb[:], ab_ps[:])
                return ab_sb, bcast

            def z_mms(a_tile, w_tile, n_ic, ow, after=None):
                """z_ps [B, ow] = a.T @ w, accumulated over n_ic chunks."""
                z_ps = pp.tile([B, ow], f32, tag="z")
                last = None
                for ic in range(n_ic):
                    mm = nc.tensor.matmul(
                        z_ps[:],
                        a_tile[:, ic * B : (ic + 1) * B],
                        w_tile[:, ic * ow : (ic + 1) * ow],
                        start=(ic == 0),
                        stop=(ic == n_ic - 1),
                    )
                    if ic == 0:
                        ordered(mm, after, "z matmuls after stats bcast")
                    last = mm
                return z_ps, last

            def tail(z_ps, n_oc, ow, bias_col, ab_sb, out_view, li, after=None):
                """transpose z back to [out, batch]; relu+bias on DVE;
                combine with alpha/beta; writes out_view [np_out, n_oc*B]."""
                np_out = min(ow, 128)
                z_sb = ap.tile([B, ow], f32, tag=f"zsb{li}")
                nc.scalar.copy(z_sb[:], z_ps[:])
                vjt = ap.tile([np_out, n_oc * B], f32, tag=f"vj{li}")
                t_sb = ap.tile([np_out, n_oc * B], f32, tag=f"t{li}")
                alpha = ab_sb[0:np_out, 0:B]
                beta = ab_sb[0:np_out, B : 2 * B]
                for oc in range(n_oc):
                    bsl = slice(oc * B, (oc + 1) * B)
                    # separate PSUM tile per oc: PE transpose-writes and
                    # DVE/ACT reads of different chunks must not serialize
                    # on Tile's per-tile bank tracking
                    zt_ps = pp.tile([np_out, B], f32, tag=f"zt{oc}")
                    tr = nc.tensor.transpose(
                        zt_ps[:],
                        z_sb[:, oc * 128 : oc * 128 + np_out],
                        teye,
                    )
                    if oc == 0:
                        ordered(tr, after, "transposes after stats bcast")
                    # relu(z + bias): alternate ACT / DVE so neither engine
                    # paces the per-oc pipeline
                    if oc % 2 == 0:
                        nc.scalar.activation(
                            vjt[:, bsl], zt_ps[:], AF.Relu,
                            bias=bias_col(oc), scale=1.0,
                        )
                    else:
                        nc.vector.tensor_scalar(
                            vjt[:, bsl], zt_ps[:], bias_col(oc), 0.0,
                            ALU.add, ALU.max,
                        )
                    # t = k1*z + beta ; out = vj*alpha + t  (per-oc so the
                    # next layer's matmul ic can start as soon as its input
                    # chunk exists)
                    nc.vector.scalar_tensor_tensor(
                        t_sb[:, bsl], zt_ps[:], k1, beta, ALU.mult, ALU.add
                    )
                    nc.vector.tensor_tensor(
                        vjt[:, bsl], vjt[:, bsl], alpha, ALU.mult
                    )
                    nc.vector.tensor_tensor(
                        out_view[:, bsl], vjt[:, bsl], t_sb[:, bsl], ALU.add
                    )

            # ---- forward chain: stats1 fills the PE while fc1 streams in;
            # later layers run stats between their z matmuls and transposes.
            ab1, bc1 = stats_ab(tx, 8, "ab1")
            z1, z1l = z_mms(tx, tw1, 8, O1, after=bc1)
            a2 = ap.tile([128, 4 * B], f32r, tag="a2")
            tail(z1, 4, O1, lambda oc: tb12[:, oc : oc + 1], ab1, a2[:], 1)

            z2, z2l = z_mms(a2[:], tw2, 4, O2)
            ab2, bc2 = stats_ab(a2[:], 4, "ab2", after_mm=z2l)
            a3 = ap.tile([128, 4 * B], f32r, tag="a3")
            tail(z2, 4, O2, lambda oc: tb12[:, 4 + oc : 5 + oc], ab2, a3[:], 2,
                 after=bc2)

            z3, z3l = z_mms(a3[:], tw3, 4, O3L)
            ab3, bc3 = stats_ab(a3[:], 4, "ab3", after_mm=z3l)
            out_sb = ap.tile([O3L, B], f32, tag="o3")
            tail(z3, 1, O3L, lambda oc: tb3, ab3, out_sb[:], 3, after=bc3)

            nc.sync.dma_start(out_d[:], out_sb[:])

    nc.compile()
    return nc


def kernel(**inputs):
    from concourse.bass_utils import run_bass_kernel_spmd

    x = np.ascontiguousarray(np.asarray(inputs["x"], dtype=np.float32))
    fc1_w = np.asarray(inputs["fc1_w"], dtype=np.float32)
    fc1_b = np.asarray(inputs["fc1_b"], dtype=np.float32)
    fc2_w = np.asarray(inputs["fc2_w"], dtype=np.float32)
    fc2_b = np.asarray(inputs["fc2_b"], dtype=np.float32)
    fc3_w = np.asarray(inputs["fc3_w"], dtype=np.float32)
    fc3_b = np.asarray(inputs["fc3_b"], dtype=np.float32)
    c1w = np.asarray(inputs["conv1_w"], dtype=np.float32)
    c1b = np.asarray(inputs["conv1_b"], dtype=np.float32)
    c2w = np.asarray(inputs["conv2_w"], dtype=np.float32)
    c2b = np.asarray(inputs["conv2_b"], dtype=np.float32)
    bn = float(np.asarray(inputs["batch_num"]).astype(np.float64))

    scale = np.float32(RATE) / np.float32(bn)
    g = (c1w.T @ c2w[0]).astype(np.float32)  # [3]
    hb = np.float32(c1b @ c2w[0] + c2b[0])
    k0 = float(scale * g[0])
    k1 = float(scale * g[1])
    k2 = float(scale * g[2])
    kb = float(scale * hb)

    key = (k0, k1, k2, kb)
    if key not in _CACHE:
        _CACHE[key] = _build(*key)
    nc = _CACHE[key]

    def pack(m, n_c, width):  # [n_c*128, width] -> [128, n_c*width]
        return np.ascontiguousarray(
            m.reshape(n_c, 128, width).transpose(1, 0, 2).reshape(128, n_c * width)
        )

    w1_h = pack(fc1_w.T, 8, O1)
    w2_h = pack(fc2_w.T, 4, O2)
    xm_h = pack(x.T, 8, B)
    # misc layout must match _build: b12 | b3 | eye | onesK | Kalpha | Kbeta
    MW = 42 + 128
    misc_h = np.zeros((128, MW), dtype=np.float32)
    misc_h[:, 0:4] = fc1_b.reshape(4, 128).T
    misc_h[:, 4:8] = fc2_b.reshape(4, 128).T
    misc_h[0:B, 9:41] = np.eye(B, dtype=np.float32)
    misc_h[:, 41] = 1.0  # ones col (K-dir sums)
    ka_m = np.zeros((96, 128), np.float16)
    ka_m[0, :] = k2
    ka_m[64, :] = 1.0  # alpha = k2*s1 + 1
    kb_m = np.zeros((96, 128), np.float16)
    kb_m[0, :] = kb
    kb_m[32, :] = k0  # beta = kb*s1 + k0*s2
    misc_h[0:96, 42:106] = ka_m.view(np.float32)
    misc_h[0:96, 106:170] = kb_m.view(np.float32)

    in_maps = []
    for c in range(N_CORES):
        w3_h = pack(fc3_w[c * O3L : (c + 1) * O3L].T, 4, O3L)
        m_h = misc_h.copy()
        m_h[0:O3L, 8] = fc3_b[c * O3L : (c + 1) * O3L]
        in_maps.append(
            dict(xm=xm_h, misc=m_h, w1t=w1_h, w2t=w2_h, w3t=w3_h)
        )

    res = run_bass_kernel_spmd(nc, in_maps, list(range(N_CORES)))
    global LAST_RESULTS
    LAST_RESULTS = res
    return np.ascontiguousarray(
        np.concatenate([res.results[c]["out"].T for c in range(N_CORES)], axis=1)
    ).astype(np.float32)


if __name__ == "__main__":
    rng = np.random.default_rng(0)

    def lin(fo, fi):
        bound = 1.0 / np.sqrt(fi)
        return (
            rng.uniform(-bound, bound, (fo, fi)).astype(np.float32),
            rng.uniform(-bound, bound, (fo,)).astype(np.float32),
        )

    fc1_w, fc1_b = lin(512, 1024)
    fc2_w, fc2_b = lin(512, 512)
    fc3_w, fc3_b = lin(256, 512)
    c1w, c1b = lin(8, 3)
    c2w, c2b = lin(1, 8)
    ins = dict(
        x=rng.standard_normal((32, 1024)).astype(np.float32),
        fc1_w=fc1_w, fc1_b=fc1_b, fc2_w=fc2_w, fc2_b=fc2_b,
        fc3_w=fc3_w, fc3_b=fc3_b,
        conv1_w=c1w, conv1_b=c1b, conv2_w=c2w, conv2_b=c2b,
        batch_num=10,
    )
    out = kernel(**ins)
    print("kernel out", out.shape, out.dtype, float(np.abs(out).max()))



# revision 6
# speedup vs baseline: 1.1137x; 1.1137x over previous
"""Trainium2 Bass kernel for nn_DiffNet (gnn_message_passing) — v2.

The reference's per-element edge MLP over (vi, W, vj) collapses
algebraically (see v1): with g = conv1_w.T @ conv2_w[0], hb =
conv1_b@conv2_w[0]+conv2_b[0], k* = scale*g*, and per-batch stats
s1 = sum_i vi, s2 = sum_i vi^2:

    out = relu(z+b) * (1 + k2*s1) + k1*z + (k0*s2 + kb*s1)

v2 changes vs the 32µs v1 (trace-driven):
  * everything on the PE dataflow is fp16 (weights, activations,
    coefficient matrices): halves the HBM stream (3.4MB -> 1.7MB,
    the kernel is memory-bound) and makes every matmul single-pass
    (1 cyc/row vs fp32's LOW_HIGH 4 cyc/row).
  * z is produced directly transposed ([out_feat, batch]) with the
    weight 128x128 block as the stationary operand, killing the
    [B,ow] PSUM tile, its SBUF copy and the 9 PE transposes of v1.
  * k1*z is rebuilt from two relus (z+b = relu(z+b) - relu(-z-b)),
    so no raw-z copy is needed; the per-(partition,oc) bias folds
    into the beta broadcast via indicator rows of the K matmul.
  * layer-3 stays in [batch, out] layout (its alpha/beta become
    per-partition columns, no PE broadcast needed) and the output
    DMA needs no host transpose.
  * per-batch stats ride tiny fp16 matmuls against a ones column;
    layer-1 stats are precomputed on host and shipped in misc.
  * DMA issue split across the two HWDGE queues (sync: weights,
    scalar: xm+misc) and w1/w2 split in halves for pipelining.

Distribution (8 cores, no collectives): fc1/fc2 replicated, fc3
sharded over its output dim (32 cols/core); full batch everywhere;
host concatenates the 8 [32,32] output shards along features.
"""

import sys

if "/opt/trn_rl_repo" not in sys.path:
    sys.path.insert(0, "/opt/trn_rl_repo")

import numpy as np


def _install_ntff_hook_shim():
    """This image's antenv lacks ``axon_hooks``; bass_utils hard-imports it
    when tracing under axon.  Provide the module and register the ctypes
    NTFF hook from trn_agent_boot so ``trace=True`` yields exec_time_ns."""
    import types

    if "antenv.axon_hooks" in sys.modules:
        return
    try:
        import antenv

        mod = types.ModuleType("antenv.axon_hooks")
        _h = [None]
        mod.set_axon_ntff_profile_hook = lambda hook: _h.__setitem__(0, hook)
        mod.get_axon_ntff_profile_hook = lambda: _h[0]
        sys.modules["antenv.axon_hooks"] = mod
        antenv.axon_hooks = mod
        from trn_agent_boot.trn_boot import _ntff_profile_via_ctypes

        mod.set_axon_ntff_profile_hook(
            _ntff_profile_via_ctypes("/opt/axon/libaxon_pjrt.so")
        )
    except Exception:
        pass


_install_ntff_hook_shim()

N_CORES = 8
B = 32
I1, O1, O2, O3 = 1024, 512, 512, 256
O3L = O3 // N_CORES  # fc3 output cols per core
RATE = 0.1
MW = 178  # misc width in f32 cols

_CACHE = {}
LAST_RESULTS = None  # BassKernelResults of the most recent run (for test.py)


def _build(k0, k1, k2, kb):
    import concourse.bacc as bacc
    import concourse.mybir as mybir
    import concourse.tile as tile
    import concourse.bass as bass

    f32 = mybir.dt.float32
    f16 = mybir.dt.float16
    AF = mybir.ActivationFunctionType
    ALU = mybir.AluOpType

    nc = bacc.Bacc(
        "TRN2", target_bir_lowering=False, debug=False, num_devices=N_CORES
    )

    xm = nc.declare_dram_parameter("xm", [128, 8 * B], f16, isOutput=False)
    misc = nc.declare_dram_parameter("misc", [128, MW], f32, isOutput=False)
    w1 = nc.declare_dram_parameter("w1b", [128, 32 * 128], f16, isOutput=False)
    w2 = nc.declare_dram_parameter("w2b", [128, 16 * 128], f16, isOutput=False)
    w3 = nc.declare_dram_parameter("w3b", [128, 4 * O3L], f16, isOutput=False)
    out_d = nc.declare_dram_parameter("out", [B, O3L], f32, isOutput=True)

    with tile.TileContext(nc) as tc:
        with (
            tc.tile_pool(name="wts", bufs=1) as wp,
            tc.tile_pool(name="act", bufs=1) as ap,
            tc.tile_pool(name="ps", bufs=1, space=bass.MemorySpace.PSUM) as pp,
        ):
            txm = wp.tile([128, 8 * B], f16, tag="xm")
            tmisc = wp.tile([128, MW], f32, tag="misc")
            tw1 = wp.tile([128, 32 * 128], f16, tag="w1")
            tw2 = wp.tile([128, 16 * 128], f16, tag="w2")
            tw3 = wp.tile([128, 4 * O3L], f16, tag="w3")

            # misc views
            tb1 = tmisc[:, 0:4]      # fc1 bias as [128, 4] (col = oc)
            tnb1 = tmisc[:, 4:8]     # negated
            tb2 = tmisc[:, 8:12]
            tnb2 = tmisc[:, 12:16]
            ts1 = tmisc[0:96, 18:82].bitcast(f16)    # s_sb1 [96, 128]
            tkbf = tmisc[:, 82:146].bitcast(f16)     # [128, 128] f16
            # kbf1 rows 0:4, kbf2 rows 32:36, kbf3 row 64 (cols 0:32),
            # ind4 rows 96:100
            tb3row = tmisc[0:1, 146:162].bitcast(f16)  # [1, 32] fc3 bias row
            tkb3row = tmisc[0:1, 162:178].bitcast(f16)  # [1, 32] -k1*b3 row

            # on-chip built coefficient matrices for the alpha/beta
            # broadcast matmuls: alpha'(j, n) = (1+k1) + k2*s1[b(n)],
            # beta'(j, n) = kb*s1 + k0*s2 - k1*bias_l[j + 128*oc(n)]
            tKa = ap.tile([96, 128], f16, tag="Ka")
            tKb1 = ap.tile([96, 128], f16, tag="Kb1")
            tKb2 = ap.tile([96, 128], f16, tag="Kb2")
            s_sb2 = ap.tile([96, 128], f16, tag="ssb2")
            onescol = ap.tile([128, 1], f16, tag="ones")
            ones1 = ap.tile([1, 32], f16, tag="ones1")

            vj = ap.tile([128, 128], f32, tag="vj")
            nvj = ap.tile([128, 128], f32, tag="nvj")
            u_sb = ap.tile([128, 128], f32, tag="u")
            t_sb = ap.tile([128, 128], f32, tag="t")
            ab_sb = ap.tile([128, 256], f32, tag="absb")
            a2 = ap.tile([128, 128], f16, tag="a2")
            a3 = ap.tile([128, 128], f16, tag="a3")
            asq = ap.tile([128, 128], f16, tag="asq")
            c3_sb = ap.tile([32, 32], f32, tag="c3sb")
            alphacol = ap.tile([32, 1], f32, tag="acol")
            bsb = ap.tile([32, 1], f32, tag="bsb")
            betacol = ap.tile([32, 1], f32, tag="bcol")
            out_sb = ap.tile([B, O3L], f32, tag="o3")

            zt = [
                pp.tile([128, B], f32, tag=f"zt{oc}", name=f"zt{oc}")
                for oc in range(4)
            ]
            ab_ps = pp.tile([128, 256], f32, tag="ab")
            s1_ps = pp.tile([32, 32], f32, tag="s1")
            s2_ps = pp.tile([32, 32], f32, tag="s2")
            z3_ps = zt[0][0:B, 0:O3L]
            c3_ps = zt[1][0:32, 0:32]

            # ---- DMA issues: weights on the sync HWDGE queue (in need
            # order), xm+misc in parallel on the scalar HWDGE queue.
            nc.scalar.dma_start(txm[:], xm[:])
            nc.scalar.dma_start(tmisc[:], misc[:])
            half1 = 16 * 128
            nc.sync.dma_start(tw1[:, 0:half1], w1[:, 0:half1])
            nc.sync.dma_start(tw1[:, half1:], w1[:, half1:])
            half2 = 8 * 128
            nc.sync.dma_start(tw2[:, 0:half2], w2[:, 0:half2])
            nc.sync.dma_start(tw2[:, half2:], w2[:, half2:])
            nc.sync.dma_start(tw3[:], w3[:])

            # ---- GpSimd setup: build K matrices & constants (junk rows
            # must be finite: zero the full tiles first).
            nc.gpsimd.memset(tKa[:], 0.0)
            nc.gpsimd.memset(tKa[0:1, :], k2)
            nc.gpsimd.memset(tKa[64:65, :], 1.0 + k1)
            nc.gpsimd.memset(tKb1[:], 0.0)
            nc.gpsimd.memset(tKb1[0:1, :], kb)
            nc.gpsimd.memset(tKb1[32:33, :], k0)
            nc.gpsimd.tensor_copy(tKb1[64:68, :], tkbf[0:4, :])
            nc.gpsimd.memset(tKb2[:], 0.0)
            nc.gpsimd.memset(tKb2[0:1, :], kb)
            nc.gpsimd.memset(tKb2[32:33, :], k0)
            nc.gpsimd.tensor_copy(tKb2[64:68, :], tkbf[32:36, :])
            nc.gpsimd.memset(onescol[:], 1.0)
            nc.gpsimd.memset(ones1[:], 1.0)
            nc.gpsimd.memset(s_sb2[:], 0.0)
            nc.gpsimd.tensor_copy(s_sb2[64:68, :], tkbf[96:100, :])

            # ---- body, emitted in dataflow order (Tile derives RAW/WAR
            # dependencies from global emission order).

            def relu_pair(oc, zsrc, bcol, nbcol):
                csl = slice(oc * B, (oc + 1) * B)
                nc.scalar.activation(vj[:, csl], zsrc, AF.Relu, bias=bcol)
                nc.scalar.activation(
                    nvj[:, csl], zsrc, AF.Relu, bias=nbcol, scale=-1.0
                )

            # broadcast 1 (host-provided stats in ts1)
            nc.tensor.matmul(ab_ps[:, 0:128], tKa[:], ts1, start=True, stop=True)
            nc.tensor.matmul(ab_ps[:, 128:256], tKb1[:], ts1, start=True, stop=True)
            nc.scalar.copy(ab_sb[:], ab_ps[:])  # ab1

            # layer 1 z^T: stationary = w1 block (oc, ic), moving = x chunk
            for oc in range(4):
                for ic in range(8):
                    blk = (oc * 8 + ic) * 128
                    nc.tensor.matmul(
                        zt[oc][:],
                        tw1[:, blk : blk + 128],
                        txm[:, ic * B : (ic + 1) * B],
                        start=(ic == 0),
                        stop=(ic == 7),
                    )
                relu_pair(oc, zt[oc][:], tb1[:, oc : oc + 1], tnb1[:, oc : oc + 1])

            # layer-1 combine -> a2
            nc.vector.tensor_tensor(u_sb[:], vj[:], ab_sb[:, 0:128], ALU.mult)
            nc.vector.scalar_tensor_tensor(
                t_sb[:], nvj[:], -k1, ab_sb[:, 128:256], ALU.mult, ALU.add
            )
            nc.vector.tensor_tensor(a2[:], u_sb[:], t_sb[:], ALU.add)

            # layer-2 stats (of a2) + s_sb2 fill
            nc.scalar.square(asq[:], a2[:])
            for ic in range(4):
                nc.tensor.matmul(
                    s1_ps[0:1, 0:B], onescol[:],
                    a2[:, ic * B : (ic + 1) * B],
                    start=(ic == 0), stop=(ic == 3),
                )
            for ic in range(4):
                nc.tensor.matmul(
                    s2_ps[0:1, 0:B], onescol[:],
                    asq[:, ic * B : (ic + 1) * B],
                    start=(ic == 0), stop=(ic == 3),
                )
            for r in range(4):
                nc.scalar.copy(s_sb2[0:1, r * B : (r + 1) * B], s1_ps[0:1, 0:B])
            for r in range(4):
                nc.scalar.copy(s_sb2[32:33, r * B : (r + 1) * B], s2_ps[0:1, 0:B])

            # layer 2 z^T (bcast2 emitted mid-way so the PE does it while
            # w2's second half streams in)
            for oc in range(4):
                if oc == 2:
                    nc.tensor.matmul(
                        ab_ps[:, 0:128], tKa[:], s_sb2[:], start=True, stop=True
                    )
                    nc.tensor.matmul(
                        ab_ps[:, 128:256], tKb2[:], s_sb2[:], start=True, stop=True
                    )
                    nc.scalar.copy(ab_sb[:], ab_ps[:])  # ab2
                for ic in range(4):
                    blk = (oc * 4 + ic) * 128
                    nc.tensor.matmul(
                        zt[oc][:],
                        tw2[:, blk : blk + 128],
                        a2[:, ic * B : (ic + 1) * B],
                        start=(ic == 0), stop=(ic == 3),
                    )
                relu_pair(oc, zt[oc][:], tb2[:, oc : oc + 1], tnb2[:, oc : oc + 1])

            # layer-2 combine -> a3
            nc.vector.tensor_tensor(u_sb[:], vj[:], ab_sb[:, 0:128], ALU.mult)
            nc.vector.scalar_tensor_tensor(
                t_sb[:], nvj[:], -k1, ab_sb[:, 128:256], ALU.mult, ALU.add
            )
            nc.vector.tensor_tensor(a3[:], u_sb[:], t_sb[:], ALU.add)

            # c3 = ones ⊗ (-k1*b3): rank-1, static inputs
            nc.tensor.matmul(c3_ps, ones1[:], tkb3row, start=True, stop=True)
            nc.scalar.copy(c3_sb[:], c3_ps)

            # layer-3 stats as columns: s1col[b] = sum_f a3[f, b]
            nc.scalar.square(asq[:], a3[:])
            for ic in range(4):
                nc.tensor.matmul(
                    s1_ps[0:32, 0:1],
                    a3[:, ic * B : (ic + 1) * B], onescol[:],
                    start=(ic == 0), stop=(ic == 3),
                )
            # alphacol = (1+k1) + k2*s1col
            nc.scalar.activation(
                alphacol[:], s1_ps[0:32, 0:1], AF.Copy, bias=1.0 + k1, scale=k2
            )
            for ic in range(4):
                nc.tensor.matmul(
                    s2_ps[0:32, 0:1],
                    asq[:, ic * B : (ic + 1) * B], onescol[:],
                    start=(ic == 0), stop=(ic == 3),
                )
            nc.scalar.activation(
                bsb[:], s2_ps[0:32, 0:1], AF.Copy, bias=0.0, scale=k0
            )
            # betacol = kb*s1col + (k0*s2col)
            nc.vector.scalar_tensor_tensor(
                betacol[:], s1_ps[0:32, 0:1], kb, bsb[:], ALU.mult, ALU.add
            )

            # layer 3 z (+ bias via rank-1): [batch, out] layout
            for ic in range(4):
                nc.tensor.matmul(
                    z3_ps,
                    a3[:, ic * B : (ic + 1) * B],
                    tw3[:, ic * O3L : (ic + 1) * O3L],
                    start=(ic == 0), stop=False,
                )
            nc.tensor.matmul(z3_ps, ones1[:], tb3row, start=False, stop=True)
            nc.scalar.activation(vj[0:B, 0:O3L], z3_ps, AF.Relu, bias=0.0)
            nc.scalar.activation(
                nvj[0:B, 0:O3L], z3_ps, AF.Relu, bias=0.0, scale=-1.0
            )

            # layer-3 combine -> out
            nc.vector.tensor_scalar_mul(
                u_sb[0:B, 0:O3L], vj[0:B, 0:O3L], alphacol[:]
            )
            nc.vector.scalar_tensor_tensor(
                t_sb[0:B, 0:O3L], nvj[0:B, 0:O3L], -k1, c3_sb[:],
                ALU.mult, ALU.add,
            )
            nc.vector.scalar_tensor_tensor(
                out_sb[:], u_sb[0:B, 0:O3L], betacol[:], t_sb[0:B, 0:O3L],
                ALU.add, ALU.add,
            )

            nc.sync.dma_start(out_d[:], out_sb[:])

    nc.compile()
    return nc


def kernel(**inputs):
    from concourse.bass_utils import run_bass_kernel_spmd

    x = np.asarray(inputs["x"], dtype=np.float32)
    fc1_w = np.asarray(inputs["fc1_w"], dtype=np.float32)
    fc1_b = np.asarray(inputs["fc1_b"], dtype=np.float32)
    fc2_w = np.asarray(inputs["fc2_w"], dtype=np.float32)
    fc2_b = np.asarray(inputs["fc2_b"], dtype=np.float32)
    fc3_w = np.asarray(inputs["fc3_w"], dtype=np.float32)
    fc3_b = np.asarray(inputs["fc3_b"], dtype=np.float32)
    c1w = np.asarray(inputs["conv1_w"], dtype=np.float32)
    c1b = np.asarray(inputs["conv1_b"], dtype=np.float32)
    c2w = np.asarray(inputs["conv2_w"], dtype=np.float32)
    c2b = np.asarray(inputs["conv2_b"], dtype=np.float32)
    bn = float(np.asarray(inputs["batch_num"]).astype(np.float64))

    scale = np.float32(RATE) / np.float32(bn)
    g = (c1w.T @ c2w[0]).astype(np.float32)  # [3]
    hb = np.float32(c1b @ c2w[0] + c2b[0])
    k0 = float(scale * g[0])
    k1 = float(scale * g[1])
    k2 = float(scale * g[2])
    kb = float(scale * hb)

    key = (k0, k1, k2, kb)
    if key not in _CACHE:
        _CACHE[key] = _build(*key)
    nc = _CACHE[key]

    f16 = np.float16

    def blocks(WT, n_ic, n_oc, ow=128):
        cols = []
        for oc in range(n_oc):
            for ic in range(n_ic):
                cols.append(WT[ic * 128 : (ic + 1) * 128, oc * ow : (oc + 1) * ow])
        return np.ascontiguousarray(np.concatenate(cols, axis=1)).astype(f16)

    w1_h = blocks(fc1_w.T, 8, 4)
    w2_h = blocks(fc2_w.T, 4, 4)
    xm_h = np.ascontiguousarray(
        x.T.reshape(8, 128, B).transpose(1, 0, 2).reshape(128, 8 * B)
    ).astype(f16)

    # host-side layer-1 stats (exact)
    s1x = x.astype(np.float64).sum(1).astype(np.float32)
    s2x = (x.astype(np.float64) ** 2).sum(1).astype(np.float32)

    misc_h = np.zeros((128, MW), dtype=np.float32)
    misc_h[:, 0:4] = fc1_b.reshape(4, 128).T
    misc_h[:, 4:8] = -fc1_b.reshape(4, 128).T
    misc_h[:, 8:12] = fc2_b.reshape(4, 128).T
    misc_h[:, 12:16] = -fc2_b.reshape(4, 128).T

    # s_sb1 [96, 128] f16: rows 0/32 = s1/s2 (4x repl), 64 = ones,
    # 65..67 = per-oc-group indicators
    ssb1 = np.zeros((96, 128), f16)
    for r in range(4):
        ssb1[0, r * B : (r + 1) * B] = s1x.astype(f16)
        ssb1[32, r * B : (r + 1) * B] = s2x.astype(f16)
    ssb1[64, :] = 1.0
    for r in (1, 2, 3):
        ssb1[64 + r, r * B : (r + 1) * B] = 1.0
    misc_h[0:96, 18:82] = ssb1.view(np.float32)

    # kbf [128, 128] f16: rows 0:4 = fold(b1), 32:36 = fold(b2),
    # 64 = -k1*b3shard (per core), 96:100 = static indicator block
    def bias_fold(bl):
        out = np.zeros((4, 128), np.float32)
        out[0] = -k1 * bl[0:128]
        for r in (1, 2, 3):
            out[r] = -k1 * (bl[r * 128 : (r + 1) * 128] - bl[0:128])
        return out

    kbf = np.zeros((128, 128), f16)
    kbf[0:4] = bias_fold(fc1_b).astype(f16)
    kbf[32:36] = bias_fold(fc2_b).astype(f16)
    kbf[96, :] = 1.0
    for r in (1, 2, 3):
        kbf[96 + r, r * B : (r + 1) * B] = 1.0

    in_maps = []
    for c in range(N_CORES):
        sh = slice(c * O3L, (c + 1) * O3L)
        w3_h = blocks(fc3_w.T[:, sh], 4, 1, ow=O3L)
        m_h = misc_h.copy()
        m_h[:, 82:146] = kbf.view(np.float32)
        m_h[0:1, 146:162] = fc3_b[sh].astype(f16).reshape(1, 32).view(np.float32)
        m_h[0:1, 162:178] = (
            (-k1 * fc3_b[sh]).astype(f16).reshape(1, 32).view(np.float32)
        )
        in_maps.append(dict(xm=xm_h, misc=m_h, w1b=w1_h, w2b=w2_h, w3b=w3_h))

    res = run_bass_kernel_spmd(nc, in_maps, list(range(N_CORES)))
    global LAST_RESULTS
    LAST_RESULTS = res
    return np.ascontiguousarray(
        np.concatenate([res.results[c]["out"] for c in range(N_CORES)], axis=1)
    ).astype(np.float32)


if __name__ == "__main__":
    rng = np.random.default_rng(0)

    def lin(fo, fi):
        bound = 1.0 / np.sqrt(fi)
        return (
            rng.uniform(-bound, bound, (fo, fi)).astype(np.float32),
            rng.uniform(-bound, bound, (fo,)).astype(np.float32),
        )

    fc1_w, fc1_b = lin(512, 1024)
    fc2_w, fc2_b = lin(512, 512)
    fc3_w, fc3_b = lin(256, 512)
    c1w, c1b = lin(8, 3)
    c2w, c2b = lin(1, 8)
    ins = dict(
        x=rng.standard_normal((32, 1024)).astype(np.float32),
        fc1_w=fc1_w, fc1_b=fc1_b, fc2_w=fc2_w, fc2_b=fc2_b,
        fc3_w=fc3_w, fc3_b=fc3_b,
        conv1_w=c1w, conv1_b=c1b, conv2_w=c2w, conv2_b=c2b,
        batch_num=10,
    )
    out = kernel(**ins)
    print("kernel out", out.shape, out.dtype, float(np.abs(out).max()))


# revision 7
# speedup vs baseline: 1.1989x; 1.0765x over previous
"""Trainium2 Bass kernel for nn_DiffNet (gnn_message_passing) — v3.

The reference's per-element edge MLP over (vi, W, vj) collapses
algebraically: with g = conv1_w.T @ conv2_w[0], hb =
conv1_b@conv2_w[0]+conv2_b[0], k* = scale*g*, and per-batch stats
s1 = sum_i vi, s2 = sum_i vi^2:

    out = relu(z+b) * (1 + k2*s1) + k1*z + (k0*s2 + kb*s1)

Kernel structure (all matmul operands fp16, PSUM f32):
  * z is produced directly transposed ([out_feat, batch]) with the
    weight 128x128 block stationary and the activation chunk moving;
    the layer bias is folded into the same PSUM accumulation group
    via a rank-1 matmul (bias_row ⊗ ones), so relu is ONE whole-tile
    ACT op per sign: vj = relu(zb), nvj = relu(-zb).
  * k1*z is rebuilt as k1*(zb - bias) = k1*(vj - nvj) - k1*bias; the
    -k1*bias lands in the beta broadcast as another rank-1 matmul.
  * alpha' = (1+k1) + k2*s1 and beta = kb*s1 + k0*s2 are broadcast
    across partitions with tiny [96,128]^T x [96,32] fp16 matmuls
    per 128-column group; the DVE combine reads them from PSUM.
  * layer-3 runs in natural [batch, out] layout: its alpha/beta are
    per-partition columns (no PE broadcast), stats come from
    lhsT=activation-chunk matmuls against a ones column, and the
    output DMA needs no host transpose.
  * layer-1 stats ship from host inside the xm tensor.
  * one HWDGE queue (sync), DMAs in consumption order: xm+stats,
    bias rows, then the fused weight wall in 4 pipelined slices.

Distribution (8 cores, no collectives): fc1/fc2 replicated, fc3
sharded over its output dim (32 cols/core); full batch everywhere;
host concatenates the 8 [32,32] output shards along features.
"""

import sys

if "/opt/trn_rl_repo" not in sys.path:
    sys.path.insert(0, "/opt/trn_rl_repo")

import numpy as np


def _install_ntff_hook_shim():
    """This image's antenv lacks ``axon_hooks``; bass_utils hard-imports it
    when tracing under axon.  Provide the module and register the ctypes
    NTFF hook from trn_agent_boot so ``trace=True`` yields exec_time_ns."""
    import types

    if "antenv.axon_hooks" in sys.modules:
        return
    try:
        import antenv

        mod = types.ModuleType("antenv.axon_hooks")
        _h = [None]
        mod.set_axon_ntff_profile_hook = lambda hook: _h.__setitem__(0, hook)
        mod.get_axon_ntff_profile_hook = lambda: _h[0]
        sys.modules["antenv.axon_hooks"] = mod
        antenv.axon_hooks = mod
        from trn_agent_boot.trn_boot import _ntff_profile_via_ctypes

        mod.set_axon_ntff_profile_hook(
            _ntff_profile_via_ctypes("/opt/axon/libaxon_pjrt.so")
        )
    except Exception:
        pass


_install_ntff_hook_shim()

N_CORES = 8
B = 32
I1, O1, O2, O3 = 1024, 512, 512, 256
O3L = O3 // N_CORES  # fc3 output cols per core
RATE = 0.1

# brow field offsets (f16 cols on partition 0)
BR_B1, BR_B2, BR_B3 = 0, 512, 1024
BR_K1B1, BR_K1B2, BR_K1B3 = 1056, 1568, 2080
BR_W = 2112
# weight wall: w1 blocks | w2 blocks | w3 chunks
WAL_W1, WAL_W2, WAL_W3, WAL_W = 0, 4096, 6144, 6272

_CACHE = {}
LAST_RESULTS = None  # BassKernelResults of the most recent run (for test.py)


def _build(k0, k1, k2, kb):
    import concourse.bacc as bacc
    import concourse.mybir as mybir
    import concourse.tile as tile
    import concourse.bass as bass

    f32 = mybir.dt.float32
    f16 = mybir.dt.float16
    AF = mybir.ActivationFunctionType
    ALU = mybir.AluOpType

    nc = bacc.Bacc(
        "TRN2", target_bir_lowering=False, debug=False, num_devices=N_CORES
    )

    xmm = nc.declare_dram_parameter("xmm", [128, 288], f16, isOutput=False)
    brow = nc.declare_dram_parameter("brow", [1, BR_W], f16, isOutput=False)
    wall = nc.declare_dram_parameter("wall", [128, WAL_W], f16, isOutput=False)
    out_d = nc.declare_dram_parameter("out", [B, O3L], f32, isOutput=True)

    with tile.TileContext(nc) as tc:
        with (
            tc.tile_pool(name="wts", bufs=1) as wp,
            tc.tile_pool(name="act", bufs=1) as ap,
            tc.tile_pool(name="ps", bufs=1, space=bass.MemorySpace.PSUM) as pp,
        ):
            txmm = wp.tile([128, 288], f16, tag="xmm")
            tbrow = wp.tile([1, BR_W], f16, tag="brow")
            twall = wp.tile([128, WAL_W], f16, tag="wall")
            ssb1 = txmm[0:96, 256:288]  # [96, 32] host layer-1 stats block

            tKa = ap.tile([96, 128], f16, tag="Ka")
            tKb = ap.tile([96, 128], f16, tag="Kb")
            s_sb2 = ap.tile([96, 32], f16, tag="ssb2")
            ones1 = ap.tile([1, 32], f16, tag="ones1")
            onescol = ap.tile([128, 1], f16, tag="ones")

            vj = ap.tile([128, 128], f32, tag="vj")
            nvj = ap.tile([128, 128], f32, tag="nvj")
            u_sb = ap.tile([128, 128], f32, tag="u")
            t_sb = ap.tile([128, 128], f32, tag="t")
            a2 = ap.tile([128, 128], f16, tag="a2")
            a3 = ap.tile([128, 128], f16, tag="a3")
            asq = ap.tile([128, 128], f16, tag="asq")
            c3_sb = ap.tile([32, 32], f32, tag="c3sb")
            alphacol = ap.tile([32, 1], f32, tag="acol")
            q_sb = ap.tile([32, 1], f32, tag="qcol")
            betacol = ap.tile([32, 1], f32, tag="bcol")
            out_sb = ap.tile([B, O3L], f32, tag="o3")

            zt_all = pp.tile([128, 128], f32, tag="zt")
            ab_ps = pp.tile([128, 256], f32, tag="ab")
            s1_ps = pp.tile([32, 32], f32, tag="s1")
            s2_ps = pp.tile([32, 32], f32, tag="s2")
            c3p = pp.tile([32, 32], f32, tag="c3p")
            z3_ps = zt_all[0:B, 0:O3L]

            # ---- DMA issues (one HWDGE queue, consumption order)
            nc.sync.dma_start(txmm[:], xmm[:])
            nc.sync.dma_start(tbrow[:], brow[:])
            for lo, hi in ((0, 2048), (2048, 4096), (4096, 5120), (5120, WAL_W)):
                nc.sync.dma_start(twall[:, lo:hi], wall[:, lo:hi])

            # ---- constants (junk rows of K matrices must be finite zeros)
            nc.gpsimd.memset(tKa[:], 0.0)
            nc.gpsimd.memset(tKa[0:1, :], k2)
            nc.gpsimd.memset(tKa[64:65, :], 1.0 + k1)
            nc.gpsimd.memset(tKb[:], 0.0)
            nc.gpsimd.memset(tKb[0:1, :], kb)
            nc.gpsimd.memset(tKb[32:33, :], k0)
            nc.gpsimd.memset(s_sb2[:], 0.0)
            nc.gpsimd.memset(s_sb2[64:65, :], 1.0)
            nc.gpsimd.memset(ones1[:], 1.0)
            nc.gpsimd.memset(onescol[:], 1.0)

            def bcast(s_rhs, k1b_off):
                """ab_ps[:, 0:128] = alpha', [:, 128:256] = beta' incl the
                -k1*bias rank-1 fold."""
                for oc in range(4):
                    csl = slice(oc * B, (oc + 1) * B)
                    nc.tensor.matmul(
                        ab_ps[:, csl], tKa[:], s_rhs, start=True, stop=True
                    )
                for oc in range(4):
                    csl = slice(128 + oc * B, 128 + (oc + 1) * B)
                    nc.tensor.matmul(
                        ab_ps[:, csl],
                        tbrow[0:1, k1b_off + oc * 128 : k1b_off + (oc + 1) * 128],
                        ones1[:],
                        start=True, stop=False,
                    )
                    nc.tensor.matmul(
                        ab_ps[:, csl], tKb[:], s_rhs, start=False, stop=True
                    )

            def zlayer(wal_off, n_ic, moving, b_off):
                """zb^T into zt_all: per oc, n_ic weight-block matmuls plus a
                rank-1 bias matmul in the same accumulation group."""
                for oc in range(4):
                    for ic in range(n_ic):
                        blk = wal_off + (oc * n_ic + ic) * 128
                        nc.tensor.matmul(
                            zt_all[:, oc * B : (oc + 1) * B],
                            twall[:, blk : blk + 128],
                            moving[:, ic * B : (ic + 1) * B],
                            start=(ic == 0), stop=False,
                        )
                    nc.tensor.matmul(
                        zt_all[:, oc * B : (oc + 1) * B],
                        tbrow[0:1, b_off + oc * 128 : b_off + (oc + 1) * 128],
                        ones1[:],
                        start=False, stop=True,
                    )

            def combine(a_next):
                """a_next = alpha'*vj - k1*nvj + beta' (alpha/beta in PSUM)."""
                nc.vector.tensor_tensor(u_sb[:], vj[:], ab_ps[:, 0:128], ALU.mult)
                nc.vector.scalar_tensor_tensor(
                    t_sb[:], nvj[:], -k1, ab_ps[:, 128:256], ALU.mult, ALU.add
                )
                nc.vector.tensor_tensor(a_next[:], u_sb[:], t_sb[:], ALU.add)

            # ---- layer 1
            bcast(ssb1, BR_K1B1)
            # c3 = ones ⊗ (-k1*b3) for the layer-3 tail (static inputs)
            nc.tensor.matmul(
                c3p[:], ones1[:], tbrow[0:1, BR_K1B3 : BR_K1B3 + 32],
                start=True, stop=True,
            )
            nc.scalar.copy(c3_sb[:], c3p[:])

            zlayer(WAL_W1, 8, txmm, BR_B1)
            nc.scalar.activation(vj[:], zt_all[:], AF.Relu, bias=0.0)
            nc.scalar.activation(nvj[:], zt_all[:], AF.Relu, bias=0.0, scale=-1.0)
            combine(a2)

            # ---- layer 2
            nc.vector.tensor_tensor(asq[:], a2[:], a2[:], ALU.mult)
            for ic in range(4):
                nc.tensor.matmul(
                    s1_ps[0:1, 0:B], onescol[:], a2[:, ic * B : (ic + 1) * B],
                    start=(ic == 0), stop=(ic == 3),
                )
            for ic in range(4):
                nc.tensor.matmul(
                    s2_ps[0:1, 0:B], onescol[:], asq[:, ic * B : (ic + 1) * B],
                    start=(ic == 0), stop=(ic == 3),
                )
            nc.scalar.copy(s_sb2[0:1, :], s1_ps[0:1, 0:B])
            nc.scalar.copy(s_sb2[32:33, :], s2_ps[0:1, 0:B])
            bcast(s_sb2[:], BR_K1B2)

            zlayer(WAL_W2, 4, a2, BR_B2)
            nc.scalar.activation(vj[:], zt_all[:], AF.Relu, bias=0.0)
            nc.scalar.activation(nvj[:], zt_all[:], AF.Relu, bias=0.0, scale=-1.0)
            combine(a3)

            # ---- layer 3 ([batch, out] layout, per-partition alpha/beta)
            nc.vector.tensor_tensor(asq[:], a3[:], a3[:], ALU.mult)
            for ic in range(4):
                nc.tensor.matmul(
                    s1_ps[0:32, 0:1], a3[:, ic * B : (ic + 1) * B], onescol[:],
                    start=(ic == 0), stop=(ic == 3),
                )
            nc.vector.tensor_scalar(
                alphacol[:], s1_ps[0:32, 0:1], k2, 1.0 + k1, ALU.mult, ALU.add
            )
            nc.vector.tensor_scalar_mul(q_sb[:], s1_ps[0:32, 0:1], kb)

            for ic in range(4):
                nc.tensor.matmul(
                    z3_ps,
                    a3[:, ic * B : (ic + 1) * B],
                    twall[:, WAL_W3 + ic * O3L : WAL_W3 + (ic + 1) * O3L],
                    start=(ic == 0), stop=False,
                )
            nc.tensor.matmul(
                z3_ps, ones1[:], tbrow[0:1, BR_B3 : BR_B3 + 32],
                start=False, stop=True,
            )
            nc.scalar.activation(vj[0:B, 0:O3L], z3_ps, AF.Relu, bias=0.0)
            nc.scalar.activation(
                nvj[0:B, 0:O3L], z3_ps, AF.Relu, bias=0.0, scale=-1.0
            )
            for ic in range(4):
                nc.tensor.matmul(
                    s2_ps[0:32, 0:1], asq[:, ic * B : (ic + 1) * B], onescol[:],
                    start=(ic == 0), stop=(ic == 3),
                )
            nc.vector.scalar_tensor_tensor(
                betacol[:], s2_ps[0:32, 0:1], k0, q_sb[:], ALU.mult, ALU.add
            )
            nc.vector.scalar_tensor_tensor(
                t_sb[0:B, 0:O3L], nvj[0:B, 0:O3L], -k1, c3_sb[:],
                ALU.mult, ALU.add,
            )
            nc.vector.tensor_scalar_mul(
                u_sb[0:B, 0:O3L], vj[0:B, 0:O3L], alphacol[:]
            )
            nc.vector.scalar_tensor_tensor(
                out_sb[:], u_sb[0:B, 0:O3L], betacol[:], t_sb[0:B, 0:O3L],
                ALU.add, ALU.add,
            )

            nc.sync.dma_start(out_d[:], out_sb[:])

    nc.compile()
    return nc


def kernel(**inputs):
    from concourse.bass_utils import run_bass_kernel_spmd

    x = np.asarray(inputs["x"], dtype=np.float32)
    fc1_w = np.asarray(inputs["fc1_w"], dtype=np.float32)
    fc1_b = np.asarray(inputs["fc1_b"], dtype=np.float32)
    fc2_w = np.asarray(inputs["fc2_w"], dtype=np.float32)
    fc2_b = np.asarray(inputs["fc2_b"], dtype=np.float32)
    fc3_w = np.asarray(inputs["fc3_w"], dtype=np.float32)
    fc3_b = np.asarray(inputs["fc3_b"], dtype=np.float32)
    c1w = np.asarray(inputs["conv1_w"], dtype=np.float32)
    c1b = np.asarray(inputs["conv1_b"], dtype=np.float32)
    c2w = np.asarray(inputs["conv2_w"], dtype=np.float32)
    c2b = np.asarray(inputs["conv2_b"], dtype=np.float32)
    bn = float(np.asarray(inputs["batch_num"]).astype(np.float64))

    scale = np.float32(RATE) / np.float32(bn)
    g = (c1w.T @ c2w[0]).astype(np.float32)  # [3]
    hb = np.float32(c1b @ c2w[0] + c2b[0])
    k0 = float(scale * g[0])
    k1 = float(scale * g[1])
    k2 = float(scale * g[2])
    kb = float(scale * hb)

    key = (k0, k1, k2, kb)
    if key not in _CACHE:
        _CACHE[key] = _build(*key)
    nc = _CACHE[key]

    f16 = np.float16

    def blocks(WT, n_ic, n_oc, ow=128):
        cols = []
        for oc in range(n_oc):
            for ic in range(n_ic):
                cols.append(WT[ic * 128 : (ic + 1) * 128, oc * ow : (oc + 1) * ow])
        return np.concatenate(cols, axis=1).astype(f16)

    w1_h = blocks(fc1_w.T, 8, 4)
    w2_h = blocks(fc2_w.T, 4, 4)

    xmm_h = np.zeros((128, 288), f16)
    xmm_h[:, 0:256] = (
        x.T.reshape(8, 128, B).transpose(1, 0, 2).reshape(128, 8 * B)
    ).astype(f16)
    s1x = x.astype(np.float64).sum(1).astype(np.float32)
    s2x = (x.astype(np.float64) ** 2).sum(1).astype(np.float32)
    xmm_h[0, 256:288] = s1x.astype(f16)
    xmm_h[32, 256:288] = s2x.astype(f16)
    xmm_h[64, 256:288] = 1.0

    brow_h = np.zeros((1, BR_W), f16)
    brow_h[0, BR_B1 : BR_B1 + 512] = fc1_b.astype(f16)
    brow_h[0, BR_B2 : BR_B2 + 512] = fc2_b.astype(f16)
    brow_h[0, BR_K1B1 : BR_K1B1 + 512] = (-k1 * fc1_b).astype(f16)
    brow_h[0, BR_K1B2 : BR_K1B2 + 512] = (-k1 * fc2_b).astype(f16)

    in_maps = []
    for c in range(N_CORES):
        sh = slice(c * O3L, (c + 1) * O3L)
        w3_h = blocks(fc3_w.T[:, sh], 4, 1, ow=O3L)
        wall_h = np.concatenate([w1_h, w2_h, w3_h], axis=1)
        br_h = brow_h.copy()
        br_h[0, BR_B3 : BR_B3 + 32] = fc3_b[sh].astype(f16)
        br_h[0, BR_K1B3 : BR_K1B3 + 32] = (-k1 * fc3_b[sh]).astype(f16)
        in_maps.append(
            dict(xmm=xmm_h, brow=br_h, wall=np.ascontiguousarray(wall_h))
        )

    res = run_bass_kernel_spmd(nc, in_maps, list(range(N_CORES)))
    global LAST_RESULTS
    LAST_RESULTS = res
    return np.ascontiguousarray(
        np.concatenate([res.results[c]["out"] for c in range(N_CORES)], axis=1)
    ).astype(np.float32)


if __name__ == "__main__":
    rng = np.random.default_rng(0)

    def lin(fo, fi):
        bound = 1.0 / np.sqrt(fi)
        return (
            rng.uniform(-bound, bound, (fo, fi)).astype(np.float32),
            rng.uniform(-bound, bound, (fo,)).astype(np.float32),
        )

    fc1_w, fc1_b = lin(512, 1024)
    fc2_w, fc2_b = lin(512, 512)
    fc3_w, fc3_b = lin(256, 512)
    c1w, c1b = lin(8, 3)
    c2w, c2b = lin(1, 8)
    ins = dict(
        x=rng.standard_normal((32, 1024)).astype(np.float32),
        fc1_w=fc1_w, fc1_b=fc1_b, fc2_w=fc2_w, fc2_b=fc2_b,
        fc3_w=fc3_w, fc3_b=fc3_b,
        conv1_w=c1w, conv1_b=c1b, conv2_w=c2w, conv2_b=c2b,
        batch_num=10,
    )
    out = kernel(**ins)
    print("kernel out", out.shape, out.dtype, float(np.abs(out).max()))
